# revision 3
# baseline (speedup 1.0000x reference)
"""Trainium2 Bass kernel for nn_DEFNet: 16-branch 1D conv (k=3..33) + bias + ReLU
+ channel-mean over x[32, 1, 262144] -> out[32, 262144].

Strategy (per core, 8 cores, 4 batch rows each):
  - Host builds a transposed sliding-window view xwinT[k, t] = xpad[64t + k]
    (k in [0,96)) plus a constant-ones row 96, so each channel-pair's conv
    AND bias is ONE matmul:
       psum[(c,p), t] = sum_k lhsT[k, 64c+p] * xwinT[k, t]
    with lhsT[k, 64c+p] = w_masked[2j+c, k-p]/16 for k<96 and
    lhsT[96, 64c+p] = b[2j+c]/16 (bias row; xwinT[96,:] = 1).
  - Per 1024-segment block, 8 pair-tiles stream through 4 PSUM buffers.
    ScalarE relus SCALAR_PAIRS tiles to bf16; VectorE runs a fused
    max(ps,0)+acc chain over VEC_PAIRS reading PSUM directly; the scalar
    tiles are tree-merged on VectorE (bf16 2x adds) and GpSimd, then the
    final [128, T] sum tile is DMA'd out position-major; the host folds the
    two 64-row halves and transposes back to natural order.
"""

import os

import numpy as np

import concourse.bass as bass
import concourse.mybir as mybir
import concourse.tile as tile
from concourse import bacc, bass_utils
from concourse.tile import TileContext

B, L = 32, 262144
NCONV, MAXK = 16, 33
NCORES = 8
ROWS = B // NCORES          # batch rows per core
P = 64                      # output positions per segment
W = 97                      # window rows (96 data + 1 bias row)
HALO = 16
T = L // P                  # segments per row (4096)

# --- tunables -------------------------------------------------------------
BLK = 1024                  # segments per block (psum tile free dim)
MMN = 512                   # matmul N (one 2KB psum bank of f32)
DT_X = mybir.dt.float16
DT_W = mybir.dt.float16
DT_E = mybir.dt.bfloat16    # relu/accumulate dtype
F32 = mybir.dt.float32

# engine split per block: scalar ACT-relus these pairs; vector chains the rest
SCALAR_PAIRS = (0, 1, 2, 3, 4)
VEC_PAIRS = (5, 6, 7)


def _support_mask():
    m = np.zeros((NCONV, MAXK), dtype=np.float32)
    c = MAXK // 2
    for i in range(1, NCONV + 1):
        m[i - 1, c - i:c + i + 1] = 1.0
    return m


def _build_lhsT(w, b):
    """[97, 8*128] f32; pair j cols j*128..(j+1)*128,
    lhsT[k, 64c+p] = wm[2j+c, k-p]/16 (k<96), lhsT[96, 64c+p] = b[2j+c]/16."""
    wm = (np.asarray(w, np.float32) * _support_mask()) / 16.0
    bs = np.asarray(b, np.float32) / 16.0
    lhsT = np.zeros((W, 8 * 128), dtype=np.float32)
    for j in range(8):
        for c in range(2):
            ch = 2 * j + c
            for p in range(P):
                lhsT[p:p + MAXK, j * 128 + c * 64 + p] = wm[ch]
            lhsT[96, j * 128 + c * 64:j * 128 + c * 64 + P] = bs[ch]
    return lhsT


def _build_nc():
    nc = bacc.Bacc(
        "TRN2",
        target_bir_lowering=False,
        debug=False,
        enable_asserts=False,
        num_devices=NCORES,
    )
    xwin = nc.dram_tensor("xwin", [ROWS * W, T], DT_X, kind="ExternalInput").ap()
    wts = nc.dram_tensor("wts", [W, 8 * 128], DT_W, kind="ExternalInput").ap()
    outH = nc.dram_tensor("outH", [ROWS * 128, T], DT_E, kind="ExternalOutput").ap()

    n_blk = T // BLK
    relu = mybir.ActivationFunctionType.Relu
    op_max, op_add = mybir.AluOpType.max, mybir.AluOpType.add
    s_pairs, v_pairs = list(SCALAR_PAIRS), list(VEC_PAIRS)
    assert len(s_pairs) == 5 and len(v_pairs) == 3, "merge tree assumes 5/3"

    with TileContext(nc) as tc:
        with (
            tc.tile_pool(name="consts", bufs=1) as cpool,
            tc.tile_pool(name="xin", bufs=3) as xpool,
            tc.tile_pool(name="psum", bufs=4, space="PSUM") as pspool,
            tc.tile_pool(name="relu", bufs=3) as rpool,
            tc.tile_pool(name="acc", bufs=3) as apool,
            tc.tile_pool(name="tmp", bufs=3) as tpool,
        ):
            w_sb = cpool.tile([W, 8 * 128], DT_W)
            nc.sync.dma_start(w_sb[:], wts[:])

            for r in range(ROWS):
                for blk in range(n_blk):
                    s0 = blk * BLK
                    x_sb = xpool.tile([W, BLK], DT_X)
                    nc.sync.dma_start(
                        x_sb[:], xwin[r * W:(r + 1) * W, s0:s0 + BLK])
                    # interleave scalar/vector pairs so both engines start
                    # early; pair-tiles stream through 4 psum buffers.
                    order = [s_pairs[0], v_pairs[0], s_pairs[1], v_pairs[1],
                             s_pairs[2], v_pairs[2], s_pairs[3], s_pairs[4]]
                    rts = {}
                    acc = None
                    m1 = m2 = None
                    for j in order:
                        lhsT = w_sb[:, j * 128:(j + 1) * 128]
                        ps = pspool.tile([128, BLK], F32)
                        for m in range(BLK // MMN):
                            nc.tensor.matmul(
                                ps[:, m * MMN:(m + 1) * MMN], lhsT,
                                x_sb[:, m * MMN:(m + 1) * MMN],
                                start=True, stop=True)
                        if j in s_pairs:
                            rt = rpool.tile([128, BLK], DT_E)
                            nc.scalar.activation(rt[:], ps[:], relu)
                            rts[j] = rt
                        elif acc is None:
                            acc = apool.tile([128, BLK], DT_E)
                            nc.vector.tensor_scalar(
                                acc[:], ps[:], 0.0, None, op_max)
                        else:
                            nacc = apool.tile([128, BLK], DT_E)
                            nc.vector.scalar_tensor_tensor(
                                nacc[:], ps[:], 0.0, acc[:], op_max, op_add)
                            acc = nacc
                        # merges as soon as their inputs exist
                        if m1 is None and s_pairs[0] in rts and s_pairs[1] in rts:
                            m1 = tpool.tile([128, BLK], DT_E)
                            nc.vector.tensor_tensor(
                                m1[:], rts[s_pairs[0]][:], rts[s_pairs[1]][:],
                                op_add)
                        if m2 is None and s_pairs[2] in rts and s_pairs[3] in rts:
                            m2 = tpool.tile([128, BLK], DT_E)
                            nc.vector.tensor_tensor(
                                m2[:], rts[s_pairs[2]][:], rts[s_pairs[3]][:],
                                op_add)
                    # gpsimd folds r4+m1, then +m2; vector joins with chain acc
                    g1 = tpool.tile([128, BLK], DT_E)
                    nc.gpsimd.tensor_tensor(
                        g1[:], rts[s_pairs[4]][:], m1[:], op_add)
                    g2 = tpool.tile([128, BLK], DT_E)
                    nc.gpsimd.tensor_tensor(g2[:], g1[:], m2[:], op_add)
                    top = tpool.tile([128, BLK], DT_E, tag="top")
                    nc.vector.tensor_tensor(top[:], g2[:], acc[:], op_add)
                    # halves fold happens on host
                    nc.sync.dma_start(
                        outH[r * 128:(r + 1) * 128, s0:s0 + BLK], top[:])
    nc.compile()
    return nc


_NC_CACHE = None


def _get_nc():
    global _NC_CACHE
    if _NC_CACHE is None:
        _NC_CACHE = _build_nc()
    return _NC_CACHE


LAST_RESULTS = None


def _install_ntff_hook():
    """Provide antenv.axon_hooks (absent on this image) so
    run_bass_kernel_spmd(trace=True) can capture NTFF profiles via the
    axon PJRT plugin's C ABI. Also stub the artifact upload (no bucket
    creds in-container)."""
    import contextlib
    import ctypes
    import sys
    import types

    try:
        from antenv.axon_hooks import get_axon_ntff_profile_hook  # noqa: F401
        return  # real module present
    except ImportError:
        pass

    so_path = "/opt/axon/libaxon_pjrt.so"
    lib = ctypes.CDLL(so_path)
    lib.axon_start_nrt_profile.argtypes = [
        ctypes.POINTER(ctypes.c_int64), ctypes.c_size_t]
    lib.axon_start_nrt_profile.restype = ctypes.c_int64
    lib.axon_stop_nrt_profile.argtypes = [ctypes.c_char_p]
    lib.axon_stop_nrt_profile.restype = ctypes.c_int64

    @contextlib.contextmanager
    def _hook(output_dir, device_ids):
        import jax
        jax.devices()
        if device_ids:
            ids = (ctypes.c_int64 * len(device_ids))(*device_ids)
            rc = lib.axon_start_nrt_profile(ids, len(device_ids))
        else:
            rc = lib.axon_start_nrt_profile(None, 0)
        if rc != 0:
            raise RuntimeError(f"axon_start_nrt_profile rc={rc}")
        try:
            yield
        finally:
            n = lib.axon_stop_nrt_profile(str(output_dir).encode())
            print(f"ntff profile: {n} file(s) -> {output_dir}")

    mod = types.ModuleType("antenv.axon_hooks")
    mod.get_axon_ntff_profile_hook = lambda: _hook
    mod.set_axon_ntff_profile_hook = lambda h: None
    sys.modules["antenv.axon_hooks"] = mod
    bass_utils.upload_artifacts = lambda tmpdir: f"file://{tmpdir}"


def host_inputs(x, w, b):
    """Build the 8 per-core input maps from the full problem inputs."""
    x = np.asarray(x, np.float32)
    xpad = np.pad(x[:, 0, :], ((0, 0), (HALO, HALO)))  # [B, L+32]
    s = xpad.strides
    np_x = mybir.dt.np(DT_X)
    xwinT = np.lib.stride_tricks.as_strided(
        xpad, shape=(B, W - 1, T), strides=(s[0], s[1], P * s[1]))

    lhsT = _build_lhsT(w, b).astype(mybir.dt.np(DT_W))

    in_maps = []
    for core in range(NCORES):
        rows = xwinT[core * ROWS:(core + 1) * ROWS]          # [4, 96, T]
        xw = np.empty((ROWS, W, T), dtype=np_x)
        xw[:, :W - 1, :] = rows
        xw[:, W - 1, :] = 1.0                                # bias row
        in_maps.append({
            "xwin": xw.reshape(ROWS * W, T),
            "wts": lhsT,
        })
    return in_maps


def kernel(x, w, b):
    global LAST_RESULTS
    in_maps = host_inputs(x, w, b)
    nc = _get_nc()
    trace = bool(os.environ.get("KERNEL_TRACE"))
    if trace:
        _install_ntff_hook()
    res = bass_utils.run_bass_kernel_spmd(
        nc, in_maps, core_ids=list(range(NCORES)), trace=trace,
        **({"trace_cores": [0]} if trace else {}),
    )
    LAST_RESULTS = res

    out = np.empty((B, L), dtype=np.float32)
    for core in range(NCORES):
        oH = res.results[core]["outH"].reshape(ROWS, 2, P, T).astype(np.float32)
        folded = oH[:, 0] + oH[:, 1]                          # [ROWS, P, T]
        for r in range(ROWS):
            out[core * ROWS + r] = folded[r].T.reshape(L)
    return out


# revision 6
# speedup vs baseline: 1.0032x; 1.0032x over previous
"""Trainium2 Bass kernel for nn_DEFNet: 16-branch 1D conv (k=3..33) + bias + ReLU
+ channel-mean over x[32, 1, 262144] -> out[32, 262144].

Strategy (per core, 8 cores, 4 batch rows each):
  - Host builds a transposed sliding-window view xwinT[k, t] = xpad[64t + k]
    (k in [0,96)) plus a constant-ones row 96, so each channel-pair's conv
    AND bias is ONE matmul:
       psum[(c,p), t] = sum_k lhsT[k, 64c+p] * xwinT[k, t]
    with lhsT[k, 64c+p] = w_masked[2j+c, k-p]/16 for k<96 and
    lhsT[96, 64c+p] = b[2j+c]/16 (bias row; xwinT[96,:] = 1).
  - Per 1024-segment block, 8 pair-tiles stream through 4 PSUM buffers
    (one N=1024 matmul each). ScalarE relus pairs 0-3 to bf16; VectorE runs
    a fused max(ps,0)+acc chain over pairs 4-7 reading PSUM directly;
    GpSimd folds the scalar tiles pairwise (r0+r1, r2+r3). The three bf16
    partials (m1, m2, acc) are DMA'd out position-major; the host sums
    them, folds the two 64-row halves, and transposes to natural order.
"""

import os

import numpy as np

import concourse.bass as bass
import concourse.mybir as mybir
import concourse.tile as tile
from concourse import bacc, bass_utils
from concourse.tile import TileContext

B, L = 32, 262144
NCONV, MAXK = 16, 33
NCORES = 8
ROWS = B // NCORES          # batch rows per core
P = 64                      # output positions per segment
W = 97                      # window rows (96 data + 1 bias row)
HALO = 16
T = L // P                  # segments per row (4096)

# --- tunables -------------------------------------------------------------
BLK = 1024                  # segments per block (2 psum banks)
MMN = 512                   # matmul N cap (one 2KB psum bank of f32)
DT_X = mybir.dt.float16
DT_W = mybir.dt.float16
DT_E = mybir.dt.bfloat16    # relu/accumulate dtype
F32 = mybir.dt.float32

SCALAR_PAIRS = (0, 1, 2, 3)   # relu'd on ScalarE, folded pairwise on GpSimd
VEC_PAIRS = (4, 5, 6, 7)      # fused max+add chain on VectorE from PSUM
NPART = 3                     # partial tiles DMA'd out per block


def _support_mask():
    m = np.zeros((NCONV, MAXK), dtype=np.float32)
    c = MAXK // 2
    for i in range(1, NCONV + 1):
        m[i - 1, c - i:c + i + 1] = 1.0
    return m


def _build_lhsT(w, b):
    """[97, 8*128] f32; pair j cols j*128..(j+1)*128,
    lhsT[k, 64c+p] = wm[2j+c, k-p]/16 (k<96), lhsT[96, 64c+p] = b[2j+c]/16."""
    wm = (np.asarray(w, np.float32) * _support_mask()) / 16.0
    bs = np.asarray(b, np.float32) / 16.0
    lhsT = np.zeros((W, 8 * 128), dtype=np.float32)
    for j in range(8):
        for c in range(2):
            ch = 2 * j + c
            for p in range(P):
                lhsT[p:p + MAXK, j * 128 + c * 64 + p] = wm[ch]
            lhsT[96, j * 128 + c * 64:j * 128 + c * 64 + P] = bs[ch]
    return lhsT


def _build_nc():
    nc = bacc.Bacc(
        "TRN2",
        target_bir_lowering=False,
        debug=False,
        enable_asserts=False,
        num_devices=NCORES,
    )
    xwin = nc.dram_tensor("xwin", [ROWS * W, T], DT_X, kind="ExternalInput").ap()
    wts = nc.dram_tensor("wts", [W, 8 * 128], DT_W, kind="ExternalInput").ap()
    outH = nc.dram_tensor(
        "outH", [ROWS * NPART * 128, T], DT_E, kind="ExternalOutput").ap()

    n_blk = T // BLK
    relu = mybir.ActivationFunctionType.Relu
    op_max, op_add = mybir.AluOpType.max, mybir.AluOpType.add
    s_pairs, v_pairs = list(SCALAR_PAIRS), list(VEC_PAIRS)

    with TileContext(nc) as tc:
        with (
            tc.tile_pool(name="consts", bufs=1) as cpool,
            tc.tile_pool(name="xin", bufs=3) as xpool,
            tc.tile_pool(name="psum", bufs=4, space="PSUM") as pspool,
            tc.tile_pool(name="relu", bufs=3) as rpool,
            tc.tile_pool(name="acc", bufs=3) as apool,
            tc.tile_pool(name="out", bufs=3) as opool,
        ):
            w_sb = cpool.tile([W, 8 * 128], DT_W)
            nc.sync.dma_start(w_sb[:], wts[:])
            # warm each compute engine's view of w_sb so later ops carry
            # fewer distinct sync waits per instruction
            warm = cpool.tile([W, 8], DT_W)
            nc.vector.tensor_copy(out=warm[:], in_=w_sb[:, 0:8])
            warm2 = cpool.tile([W, 8], DT_W)
            nc.gpsimd.tensor_copy(out=warm2[:], in_=w_sb[:, 0:8])
            warm3 = cpool.tile([W, 8], DT_W)
            nc.scalar.copy(warm3[:], w_sb[:, 0:8])

            for r in range(ROWS):
                for blk in range(n_blk):
                    s0 = blk * BLK
                    x_sb = xpool.tile([W, BLK], DT_X)
                    nc.sync.dma_start(
                        x_sb[:], xwin[r * W:(r + 1) * W, s0:s0 + BLK])
                    # interleave scalar/vector pairs so both engines start
                    # early; pair-tiles stream through 4 psum buffers.
                    order = [s_pairs[0], v_pairs[0], s_pairs[1], v_pairs[1],
                             s_pairs[2], v_pairs[2], s_pairs[3], v_pairs[3]]
                    rts = {}
                    acc = None
                    m1 = m2 = None
                    for j in order:
                        lhsT = w_sb[:, j * 128:(j + 1) * 128]
                        ps = pspool.tile([128, BLK], F32)
                        for m in range(BLK // MMN):
                            nc.tensor.matmul(
                                ps[:, m * MMN:(m + 1) * MMN], lhsT,
                                x_sb[:, m * MMN:(m + 1) * MMN],
                                start=True, stop=True)
                        if j in s_pairs:
                            rt = rpool.tile([128, BLK], DT_E)
                            nc.scalar.activation(rt[:], ps[:], relu)
                            rts[j] = rt
                        elif acc is None:
                            acc = apool.tile([128, BLK], DT_E)
                            nc.vector.tensor_scalar(
                                acc[:], ps[:], 0.0, None, op_max)
                        else:
                            nacc = apool.tile([128, BLK], DT_E)
                            nc.vector.scalar_tensor_tensor(
                                nacc[:], ps[:], 0.0, acc[:], op_max, op_add)
                            acc = nacc
                        # gpsimd folds as soon as scalar tile pairs exist
                        if m1 is None and s_pairs[0] in rts and s_pairs[1] in rts:
                            m1 = opool.tile([128, BLK], DT_E)
                            nc.gpsimd.tensor_tensor(
                                m1[:], rts[s_pairs[0]][:], rts[s_pairs[1]][:],
                                op_add)
                        if m2 is None and s_pairs[2] in rts and s_pairs[3] in rts:
                            m2 = opool.tile([128, BLK], DT_E)
                            nc.gpsimd.tensor_tensor(
                                m2[:], rts[s_pairs[2]][:], rts[s_pairs[3]][:],
                                op_add)
                    # three partials out; host sums + folds halves
                    base = r * NPART * 128
                    nc.sync.dma_start(
                        outH[base:base + 128, s0:s0 + BLK], m1[:])
                    nc.sync.dma_start(
                        outH[base + 128:base + 256, s0:s0 + BLK], m2[:])
                    nc.sync.dma_start(
                        outH[base + 256:base + 384, s0:s0 + BLK], acc[:])
    nc.compile()
    return nc


_NC_CACHE = None


def _get_nc():
    global _NC_CACHE
    if _NC_CACHE is None:
        _NC_CACHE = _build_nc()
    return _NC_CACHE


LAST_RESULTS = None


def _install_ntff_hook():
    """Provide antenv.axon_hooks (absent on this image) so
    run_bass_kernel_spmd(trace=True) can capture NTFF profiles via the
    axon PJRT plugin's C ABI. Also stub the artifact upload (no bucket
    creds in-container)."""
    import contextlib
    import ctypes
    import sys
    import types

    try:
        from antenv.axon_hooks import get_axon_ntff_profile_hook  # noqa: F401
        return  # real module present
    except ImportError:
        pass

    so_path = "/opt/axon/libaxon_pjrt.so"
    lib = ctypes.CDLL(so_path)
    lib.axon_start_nrt_profile.argtypes = [
        ctypes.POINTER(ctypes.c_int64), ctypes.c_size_t]
    lib.axon_start_nrt_profile.restype = ctypes.c_int64
    lib.axon_stop_nrt_profile.argtypes = [ctypes.c_char_p]
    lib.axon_stop_nrt_profile.restype = ctypes.c_int64

    @contextlib.contextmanager
    def _hook(output_dir, device_ids):
        import jax
        jax.devices()
        if device_ids:
            ids = (ctypes.c_int64 * len(device_ids))(*device_ids)
            rc = lib.axon_start_nrt_profile(ids, len(device_ids))
        else:
            rc = lib.axon_start_nrt_profile(None, 0)
        if rc != 0:
            raise RuntimeError(f"axon_start_nrt_profile rc={rc}")
        try:
            yield
        finally:
            n = lib.axon_stop_nrt_profile(str(output_dir).encode())
            print(f"ntff profile: {n} file(s) -> {output_dir}")

    mod = types.ModuleType("antenv.axon_hooks")
    mod.get_axon_ntff_profile_hook = lambda: _hook
    mod.set_axon_ntff_profile_hook = lambda h: None
    sys.modules["antenv.axon_hooks"] = mod
    bass_utils.upload_artifacts = lambda tmpdir: f"file://{tmpdir}"


def host_inputs(x, w, b):
    """Build the 8 per-core input maps from the full problem inputs."""
    x = np.asarray(x, np.float32)
    xpad = np.pad(x[:, 0, :], ((0, 0), (HALO, HALO)))  # [B, L+32]
    s = xpad.strides
    np_x = mybir.dt.np(DT_X)
    xwinT = np.lib.stride_tricks.as_strided(
        xpad, shape=(B, W - 1, T), strides=(s[0], s[1], P * s[1]))

    lhsT = _build_lhsT(w, b).astype(mybir.dt.np(DT_W))

    in_maps = []
    for core in range(NCORES):
        rows = xwinT[core * ROWS:(core + 1) * ROWS]          # [4, 96, T]
        xw = np.empty((ROWS, W, T), dtype=np_x)
        xw[:, :W - 1, :] = rows
        xw[:, W - 1, :] = 1.0                                # bias row
        in_maps.append({
            "xwin": xw.reshape(ROWS * W, T),
            "wts": lhsT,
        })
    return in_maps


def kernel(x, w, b):
    global LAST_RESULTS
    in_maps = host_inputs(x, w, b)
    nc = _get_nc()
    trace = bool(os.environ.get("KERNEL_TRACE"))
    if trace:
        _install_ntff_hook()
    res = bass_utils.run_bass_kernel_spmd(
        nc, in_maps, core_ids=list(range(NCORES)), trace=trace,
        **({"trace_cores": [0]} if trace else {}),
    )
    LAST_RESULTS = res

    out = np.empty((B, L), dtype=np.float32)
    for core in range(NCORES):
        oH = res.results[core]["outH"].reshape(
            ROWS, NPART, 2, P, T).astype(np.float32)
        folded = oH.sum(axis=(1, 2))                          # [ROWS, P, T]
        for r in range(ROWS):
            out[core * ROWS + r] = folded[r].T.reshape(L)
    return out


# revision 8
# speedup vs baseline: 1.0349x; 1.0316x over previous
"""Trainium2 Bass kernel for nn_DEFNet: 16-branch 1D conv (k=3..33) + bias + ReLU
+ channel-mean over x[32, 1, 262144] -> out[32, 262144].

Strategy (per core, 8 cores, 4 batch rows each):
  - Host builds a transposed sliding-window view xwinT[k, t] = xpad[64t + k]
    (k in [0,96)) plus a constant-ones row 96, so each channel-pair's conv
    AND bias is ONE matmul:
       psum[(c,p), t] = sum_k lhsT[k, 64c+p] * xwinT[k, t]
    with lhsT[k, 64c+p] = w_masked[2j+c, k-p]/16 for k<96 and
    lhsT[96, 64c+p] = b[2j+c]/16 (bias row; xwinT[96,:] = 1).
  - Per 1024-segment block, 8 pair-tiles stream through 4 PSUM buffers
    (2 N=512 matmuls each). ScalarE relus pairs 0-4 to bf16; VectorE runs a
    fused max(ps,0)+acc chain over pairs 5-7 reading PSUM directly, seeded
    with scalar tile r4 (STT src1), and pair-folds r0+r1 / r2+r3 (bf16 2x).
    The three bf16 partials (m1, m2, chain) land side by side in one out
    tile, DMA'd with a single transfer per block; the host sums the
    partials, folds the two 64-row halves, and transposes to natural order.
"""

import os

import numpy as np

import concourse.bass as bass
import concourse.mybir as mybir
import concourse.tile as tile
from concourse import bacc, bass_utils
from concourse.tile import TileContext

B, L = 32, 262144
NCONV, MAXK = 16, 33
NCORES = 8
ROWS = B // NCORES          # batch rows per core
P = 64                      # output positions per segment
W = 97                      # window rows (96 data + 1 bias row)
HALO = 16
T = L // P                  # segments per row (4096)

# --- tunables -------------------------------------------------------------
BLK = 1024                  # segments per block (2 psum banks)
MMN = 512                   # matmul N cap (one 2KB psum bank of f32)
XBLK = 2048                 # segments per x-in DMA (2 blocks)
DT_X = mybir.dt.float16
DT_W = mybir.dt.float16
DT_E = mybir.dt.bfloat16    # relu/accumulate dtype
F32 = mybir.dt.float32

NPART = 3                   # partial tiles per block in the combined out tile


def _support_mask():
    m = np.zeros((NCONV, MAXK), dtype=np.float32)
    c = MAXK // 2
    for i in range(1, NCONV + 1):
        m[i - 1, c - i:c + i + 1] = 1.0
    return m


def _build_lhsT(w, b):
    """[97, 8*128] f32; pair j cols j*128..(j+1)*128,
    lhsT[k, 64c+p] = wm[2j+c, k-p]/16 (k<96), lhsT[96, 64c+p] = b[2j+c]/16."""
    wm = (np.asarray(w, np.float32) * _support_mask()) / 16.0
    bs = np.asarray(b, np.float32) / 16.0
    lhsT = np.zeros((W, 8 * 128), dtype=np.float32)
    for j in range(8):
        for c in range(2):
            ch = 2 * j + c
            for p in range(P):
                lhsT[p:p + MAXK, j * 128 + c * 64 + p] = wm[ch]
            lhsT[96, j * 128 + c * 64:j * 128 + c * 64 + P] = bs[ch]
    return lhsT


def _build_nc():
    nc = bacc.Bacc(
        "TRN2",
        target_bir_lowering=False,
        debug=False,
        enable_asserts=False,
        num_devices=NCORES,
    )
    xwin = nc.dram_tensor("xwin", [ROWS * W, T], DT_X, kind="ExternalInput").ap()
    wts = nc.dram_tensor("wts", [W, 8 * 128], DT_W, kind="ExternalInput").ap()
    outH = nc.dram_tensor(
        "outH", [ROWS * 128, NPART * T], DT_E, kind="ExternalOutput").ap()

    n_blk = T // BLK
    relu = mybir.ActivationFunctionType.Relu
    op_max, op_add = mybir.AluOpType.max, mybir.AluOpType.add

    with TileContext(nc) as tc:
        with (
            tc.tile_pool(name="consts", bufs=1) as cpool,
            tc.tile_pool(name="xin", bufs=3) as xpool,
            tc.tile_pool(name="psum", bufs=4, space="PSUM") as pspool,
            tc.tile_pool(name="relu", bufs=3) as rpool,
            tc.tile_pool(name="acc", bufs=3) as apool,
            tc.tile_pool(name="out", bufs=3) as opool,
        ):
            w_sb = cpool.tile([W, 8 * 128], DT_W)
            nc.sync.dma_start(w_sb[:], wts[:])
            # warm scalar/vector views of w_sb so later ops carry fewer
            # distinct sync waits per instruction
            warm = cpool.tile([W, 8], DT_W)
            nc.vector.tensor_copy(out=warm[:], in_=w_sb[:, 0:8])
            warm3 = cpool.tile([W, 8], DT_W)
            nc.scalar.copy(warm3[:], w_sb[:, 0:8])

            for r in range(ROWS):
                for xb in range(T // XBLK):
                    x_sb = xpool.tile([W, XBLK], DT_X)
                    nc.sync.dma_start(
                        x_sb[:], xwin[r * W:(r + 1) * W,
                                      xb * XBLK:(xb + 1) * XBLK])
                    for sub in range(XBLK // BLK):
                        blk = xb * (XBLK // BLK) + sub
                        s0 = blk * BLK
                        xs = x_sb[:, sub * BLK:(sub + 1) * BLK]
                        ot = opool.tile([128, NPART * BLK], DT_E, tag="ot")
                        # pair 4 first (seeds the vector chain), then
                        # alternate vector/scalar pairs
                        order = [4, 5, 0, 6, 1, 7, 2, 3]
                        rts = {}
                        acc = None
                        for j in order:
                            lhsT = w_sb[:, j * 128:(j + 1) * 128]
                            ps = pspool.tile([128, BLK], F32)
                            for m in range(BLK // MMN):
                                nc.tensor.matmul(
                                    ps[:, m * MMN:(m + 1) * MMN], lhsT,
                                    xs[:, m * MMN:(m + 1) * MMN],
                                    start=True, stop=True)
                            if j < 5:
                                rt = rpool.tile([128, BLK], DT_E)
                                nc.scalar.activation(rt[:], ps[:], relu)
                                rts[j] = rt
                            else:
                                dst = (ot[:, 2 * BLK:3 * BLK] if j == 7
                                       else None)
                                if acc is None:
                                    nacc = apool.tile(
                                        [128, BLK], DT_E, tag="chain")
                                    nc.vector.scalar_tensor_tensor(
                                        nacc[:], ps[:], 0.0, rts[4][:],
                                        op_max, op_add)
                                    acc = nacc
                                elif dst is None:
                                    nacc = apool.tile(
                                        [128, BLK], DT_E, tag="chain")
                                    nc.vector.scalar_tensor_tensor(
                                        nacc[:], ps[:], 0.0, acc[:],
                                        op_max, op_add)
                                    acc = nacc
                                else:
                                    nc.vector.scalar_tensor_tensor(
                                        dst, ps[:], 0.0, acc[:],
                                        op_max, op_add)
                            if j == 1:
                                nc.vector.tensor_tensor(
                                    ot[:, 0:BLK], rts[0][:], rts[1][:],
                                    op_add)
                            elif j == 3:
                                nc.vector.tensor_tensor(
                                    ot[:, BLK:2 * BLK], rts[2][:], rts[3][:],
                                    op_add)
                        nc.sync.dma_start(
                            outH[r * 128:(r + 1) * 128,
                                 NPART * s0:NPART * (s0 + BLK)], ot[:])
    nc.compile()
    return nc


_NC_CACHE = None


def _get_nc():
    global _NC_CACHE
    if _NC_CACHE is None:
        _NC_CACHE = _build_nc()
    return _NC_CACHE


LAST_RESULTS = None


def _install_ntff_hook():
    """Provide antenv.axon_hooks (absent on this image) so
    run_bass_kernel_spmd(trace=True) can capture NTFF profiles via the
    axon PJRT plugin's C ABI. Also stub the artifact upload (no bucket
    creds in-container)."""
    import contextlib
    import ctypes
    import sys
    import types

    try:
        from antenv.axon_hooks import get_axon_ntff_profile_hook  # noqa: F401
        return  # real module present
    except ImportError:
        pass

    so_path = "/opt/axon/libaxon_pjrt.so"
    lib = ctypes.CDLL(so_path)
    lib.axon_start_nrt_profile.argtypes = [
        ctypes.POINTER(ctypes.c_int64), ctypes.c_size_t]
    lib.axon_start_nrt_profile.restype = ctypes.c_int64
    lib.axon_stop_nrt_profile.argtypes = [ctypes.c_char_p]
    lib.axon_stop_nrt_profile.restype = ctypes.c_int64

    @contextlib.contextmanager
    def _hook(output_dir, device_ids):
        import jax
        jax.devices()
        if device_ids:
            ids = (ctypes.c_int64 * len(device_ids))(*device_ids)
            rc = lib.axon_start_nrt_profile(ids, len(device_ids))
        else:
            rc = lib.axon_start_nrt_profile(None, 0)
        if rc != 0:
            raise RuntimeError(f"axon_start_nrt_profile rc={rc}")
        try:
            yield
        finally:
            n = lib.axon_stop_nrt_profile(str(output_dir).encode())
            print(f"ntff profile: {n} file(s) -> {output_dir}")

    mod = types.ModuleType("antenv.axon_hooks")
    mod.get_axon_ntff_profile_hook = lambda: _hook
    mod.set_axon_ntff_profile_hook = lambda h: None
    sys.modules["antenv.axon_hooks"] = mod
    bass_utils.upload_artifacts = lambda tmpdir: f"file://{tmpdir}"


def host_inputs(x, w, b):
    """Build the 8 per-core input maps from the full problem inputs."""
    x = np.asarray(x, np.float32)
    xpad = np.pad(x[:, 0, :], ((0, 0), (HALO, HALO)))  # [B, L+32]
    s = xpad.strides
    np_x = mybir.dt.np(DT_X)
    xwinT = np.lib.stride_tricks.as_strided(
        xpad, shape=(B, W - 1, T), strides=(s[0], s[1], P * s[1]))

    lhsT = _build_lhsT(w, b).astype(mybir.dt.np(DT_W))

    in_maps = []
    for core in range(NCORES):
        rows = xwinT[core * ROWS:(core + 1) * ROWS]          # [4, 96, T]
        xw = np.empty((ROWS, W, T), dtype=np_x)
        xw[:, :W - 1, :] = rows
        xw[:, W - 1, :] = 1.0                                # bias row
        in_maps.append({
            "xwin": xw.reshape(ROWS * W, T),
            "wts": lhsT,
        })
    return in_maps


def kernel(x, w, b):
    global LAST_RESULTS
    in_maps = host_inputs(x, w, b)
    nc = _get_nc()
    trace = bool(os.environ.get("KERNEL_TRACE"))
    if trace:
        _install_ntff_hook()
    res = bass_utils.run_bass_kernel_spmd(
        nc, in_maps, core_ids=list(range(NCORES)), trace=trace,
        **({"trace_cores": [0]} if trace else {}),
    )
    LAST_RESULTS = res

    n_blk = T // BLK
    out = np.empty((B, L), dtype=np.float32)
    for core in range(NCORES):
        # outH rows: [ROWS, 2, P]; cols: [n_blk, NPART, BLK]
        oH = res.results[core]["outH"].reshape(
            ROWS, 2, P, n_blk, NPART, BLK).astype(np.float32)
        folded = oH.sum(axis=(1, 4))                          # [ROWS, P, n_blk, BLK]
        for r in range(ROWS):
            # position = (blk*BLK + t)*64 + p  ->  [n_blk, BLK, P] order
            out[core * ROWS + r] = folded[r].transpose(1, 2, 0).reshape(L)
    return out


# revision 9
# speedup vs baseline: 1.7316x; 1.6733x over previous
"""Trainium2 Bass kernel for nn_DEFNet: 16-branch 1D conv (k=3..33) + bias + ReLU
+ channel-mean over x[32, 1, 262144] -> out[32, 262144].

Strategy (per core, 8 cores, 4 batch rows each):
  - Host builds a transposed sliding-window view xwinT[k, t] = xpad[64t + k]
    (k in [0,96)) so each channel-pair's conv is ONE matmul:
       psum[(c,p), t] = sum_k lhsT[k, 64c+p] * xwinT[k, t]
    with lhsT[k, 64c+p] = w_masked[2j+c, k-p]/16 (mean folded into weights).
    96 rows keeps every DMA a multiple of 16 partitions so descriptors
    spread across all 16 SDMA engines (a 97th bias row serializes them).
  - Per 1024-segment block, 8 pair-tiles stream through 4 PSUM buffers
    (2 N=512 matmuls each). ScalarE relus pairs 0-4 to bf16 (bias fused);
    VectorE runs a fused max(ps,-b)+acc chain over pairs 5-7 reading PSUM
    directly, seeded with scalar tile r4 (STT src1), and pair-folds r0+r1 /
    r2+r3 (bf16 2x adds). The three bf16 partials (m1, m2, chain) land side
    by side in one out tile, DMA'd with a single transfer per block; the
    host sums partials, adds the chain-bias constant, folds the two 64-row
    halves, and transposes to natural order.
"""

import os

import numpy as np

import concourse.bass as bass
import concourse.mybir as mybir
import concourse.tile as tile
from concourse import bacc, bass_utils
from concourse.tile import TileContext

B, L = 32, 262144
NCONV, MAXK = 16, 33
NCORES = 8
ROWS = B // NCORES          # batch rows per core
P = 64                      # output positions per segment
W = 96                      # window rows (matmul contraction dim)
HALO = 16
T = L // P                  # segments per row (4096)

# --- tunables -------------------------------------------------------------
BLK = 1024                  # segments per block (2 psum banks)
MMN = 512                   # matmul N cap (one 2KB psum bank of f32)
XBLK = 2048                 # segments per x-in DMA (2 blocks)
DT_X = mybir.dt.float16
DT_W = mybir.dt.float16
DT_E = mybir.dt.bfloat16    # relu/accumulate dtype
F32 = mybir.dt.float32

NPART = 3                   # partial tiles per block in the combined out tile
CHAIN_PAIRS = (5, 6, 7)     # fused max-add chain on VectorE (reads PSUM)


def _support_mask():
    m = np.zeros((NCONV, MAXK), dtype=np.float32)
    c = MAXK // 2
    for i in range(1, NCONV + 1):
        m[i - 1, c - i:c + i + 1] = 1.0
    return m


def _build_lhsT(w):
    """[96, 8*128] f32; pair j cols j*128..(j+1)*128,
    lhsT[k, 64c+p] = wm[2j+c, k-p]/16."""
    wm = (np.asarray(w, np.float32) * _support_mask()) / 16.0
    lhsT = np.zeros((W, 8 * 128), dtype=np.float32)
    for j in range(8):
        for c in range(2):
            ch = 2 * j + c
            for p in range(P):
                lhsT[p:p + MAXK, j * 128 + c * 64 + p] = wm[ch]
    return lhsT


def _build_nc():
    nc = bacc.Bacc(
        "TRN2",
        target_bir_lowering=False,
        debug=False,
        enable_asserts=False,
        num_devices=NCORES,
    )
    xwin = nc.dram_tensor("xwin", [ROWS * W, T], DT_X, kind="ExternalInput").ap()
    wts = nc.dram_tensor("wts", [W, 8 * 128], DT_W, kind="ExternalInput").ap()
    btab = nc.dram_tensor("btab", [128, 16], F32, kind="ExternalInput").ap()
    outH = nc.dram_tensor(
        "outH", [ROWS * 128, NPART * T], DT_E, kind="ExternalOutput").ap()

    n_blk = T // BLK
    relu = mybir.ActivationFunctionType.Relu
    op_max, op_add = mybir.AluOpType.max, mybir.AluOpType.add

    with TileContext(nc) as tc:
        with (
            tc.tile_pool(name="consts", bufs=1) as cpool,
            tc.tile_pool(name="xin", bufs=3) as xpool,
            tc.tile_pool(name="psum", bufs=4, space="PSUM") as pspool,
            tc.tile_pool(name="relu", bufs=3) as rpool,
            tc.tile_pool(name="acc", bufs=3) as apool,
            tc.tile_pool(name="out", bufs=3) as opool,
        ):
            w_sb = cpool.tile([W, 8 * 128], DT_W)
            nc.sync.dma_start(w_sb[:], wts[:])
            b_sb = cpool.tile([128, 16], F32)
            nc.sync.dma_start(b_sb[:], btab[:])
            # cols 0..7: +b per pair (scalar ACT bias); 8..15: -b (chain)

            # warm scalar/vector views of the const tiles so later ops carry
            # fewer distinct sync waits per instruction
            warm = cpool.tile([128, 16], F32)
            nc.vector.tensor_copy(out=warm[:], in_=b_sb[:])
            warm3 = cpool.tile([128, 16], F32)
            nc.scalar.copy(warm3[:], b_sb[:])

            for r in range(ROWS):
                for xb in range(T // XBLK):
                    x_sb = xpool.tile([W, XBLK], DT_X)
                    nc.sync.dma_start(
                        x_sb[:], xwin[r * W:(r + 1) * W,
                                      xb * XBLK:(xb + 1) * XBLK])
                    for sub in range(XBLK // BLK):
                        blk = xb * (XBLK // BLK) + sub
                        s0 = blk * BLK
                        xs = x_sb[:, sub * BLK:(sub + 1) * BLK]
                        ot = opool.tile([128, NPART * BLK], DT_E, tag="ot")
                        # pair 4 first (seeds the vector chain), then
                        # alternate vector/scalar pairs
                        order = [4, 5, 0, 6, 1, 7, 2, 3]
                        rts = {}
                        acc = None
                        for j in order:
                            lhsT = w_sb[:, j * 128:(j + 1) * 128]
                            ps = pspool.tile([128, BLK], F32)
                            for m in range(BLK // MMN):
                                nc.tensor.matmul(
                                    ps[:, m * MMN:(m + 1) * MMN], lhsT,
                                    xs[:, m * MMN:(m + 1) * MMN],
                                    start=True, stop=True)
                            if j not in CHAIN_PAIRS:
                                rt = rpool.tile([128, BLK], DT_E)
                                nc.scalar.activation(
                                    rt[:], ps[:], relu, bias=b_sb[:, j:j + 1])
                                rts[j] = rt
                            elif acc is None:
                                nacc = apool.tile([128, BLK], DT_E, tag="ch")
                                nc.vector.scalar_tensor_tensor(
                                    nacc[:], ps[:], b_sb[:, 8 + j:9 + j],
                                    rts[4][:], op_max, op_add)
                                acc = nacc
                            elif j != 7:
                                nacc = apool.tile([128, BLK], DT_E, tag="ch")
                                nc.vector.scalar_tensor_tensor(
                                    nacc[:], ps[:], b_sb[:, 8 + j:9 + j],
                                    acc[:], op_max, op_add)
                                acc = nacc
                            else:
                                nc.vector.scalar_tensor_tensor(
                                    ot[:, 2 * BLK:3 * BLK], ps[:],
                                    b_sb[:, 8 + j:9 + j], acc[:],
                                    op_max, op_add)
                            if j == 1:
                                nc.vector.tensor_tensor(
                                    ot[:, 0:BLK], rts[0][:], rts[1][:],
                                    op_add)
                            elif j == 3:
                                nc.vector.tensor_tensor(
                                    ot[:, BLK:2 * BLK], rts[2][:], rts[3][:],
                                    op_add)
                        nc.sync.dma_start(
                            outH[r * 128:(r + 1) * 128,
                                 NPART * s0:NPART * (s0 + BLK)], ot[:])
    nc.compile()
    return nc


_NC_CACHE = None


def _get_nc():
    global _NC_CACHE
    if _NC_CACHE is None:
        _NC_CACHE = _build_nc()
    return _NC_CACHE


LAST_RESULTS = None


def _install_ntff_hook():
    """Provide antenv.axon_hooks (absent on this image) so
    run_bass_kernel_spmd(trace=True) can capture NTFF profiles via the
    axon PJRT plugin's C ABI. Also stub the artifact upload (no bucket
    creds in-container)."""
    import contextlib
    import ctypes
    import sys
    import types

    try:
        from antenv.axon_hooks import get_axon_ntff_profile_hook  # noqa: F401
        return  # real module present
    except ImportError:
        pass

    so_path = "/opt/axon/libaxon_pjrt.so"
    lib = ctypes.CDLL(so_path)
    lib.axon_start_nrt_profile.argtypes = [
        ctypes.POINTER(ctypes.c_int64), ctypes.c_size_t]
    lib.axon_start_nrt_profile.restype = ctypes.c_int64
    lib.axon_stop_nrt_profile.argtypes = [ctypes.c_char_p]
    lib.axon_stop_nrt_profile.restype = ctypes.c_int64

    @contextlib.contextmanager
    def _hook(output_dir, device_ids):
        import jax
        jax.devices()
        if device_ids:
            ids = (ctypes.c_int64 * len(device_ids))(*device_ids)
            rc = lib.axon_start_nrt_profile(ids, len(device_ids))
        else:
            rc = lib.axon_start_nrt_profile(None, 0)
        if rc != 0:
            raise RuntimeError(f"axon_start_nrt_profile rc={rc}")
        try:
            yield
        finally:
            n = lib.axon_stop_nrt_profile(str(output_dir).encode())
            print(f"ntff profile: {n} file(s) -> {output_dir}")

    mod = types.ModuleType("antenv.axon_hooks")
    mod.get_axon_ntff_profile_hook = lambda: _hook
    mod.set_axon_ntff_profile_hook = lambda h: None
    sys.modules["antenv.axon_hooks"] = mod
    bass_utils.upload_artifacts = lambda tmpdir: f"file://{tmpdir}"


def host_inputs(x, w, b):
    """Build the 8 per-core input maps from the full problem inputs."""
    x = np.asarray(x, np.float32)
    xpad = np.pad(x[:, 0, :], ((0, 0), (HALO, HALO)))  # [B, L+32]
    s = xpad.strides
    np_x = mybir.dt.np(DT_X)
    xwinT = np.lib.stride_tricks.as_strided(
        xpad, shape=(B, W, T), strides=(s[0], s[1], P * s[1]))

    lhsT = _build_lhsT(w).astype(mybir.dt.np(DT_W))
    bsc = np.asarray(b, np.float32) / 16.0
    btab = np.zeros((128, 16), dtype=np.float32)
    for j in range(8):
        col = np.concatenate(
            [np.full(P, bsc[2 * j]), np.full(P, bsc[2 * j + 1])])
        btab[:, j] = col
        btab[:, 8 + j] = -col

    in_maps = []
    for core in range(NCORES):
        rows = xwinT[core * ROWS:(core + 1) * ROWS]          # [4, 96, T]
        in_maps.append({
            "xwin": np.ascontiguousarray(rows, dtype=np_x).reshape(ROWS * W, T),
            "wts": lhsT,
            "btab": btab,
        })
    return in_maps


def kernel(x, w, b):
    global LAST_RESULTS
    in_maps = host_inputs(x, w, b)
    nc = _get_nc()
    trace = bool(os.environ.get("KERNEL_TRACE"))
    if trace:
        _install_ntff_hook()
    res = bass_utils.run_bass_kernel_spmd(
        nc, in_maps, core_ids=list(range(NCORES)), trace=trace,
        **({"trace_cores": [0]} if trace else {}),
    )
    LAST_RESULTS = res

    # chain pairs accumulate relu(y)-b; add back sum of their biases
    bsc = np.asarray(b, np.float32) / 16.0
    cb = sum(float(bsc[2 * j] + bsc[2 * j + 1]) for j in CHAIN_PAIRS)
    n_blk = T // BLK
    out = np.empty((B, L), dtype=np.float32)
    for core in range(NCORES):
        # outH rows: [ROWS, 2, P]; cols: [n_blk, NPART, BLK]
        oH = res.results[core]["outH"].reshape(
            ROWS, 2, P, n_blk, NPART, BLK).astype(np.float32)
        folded = oH.sum(axis=(1, 4)) + cb                     # [ROWS, P, n_blk, BLK]
        for r in range(ROWS):
            # position = (blk*BLK + t)*64 + p  ->  [n_blk, BLK, P] order
            out[core * ROWS + r] = folded[r].transpose(1, 2, 0).reshape(L)
    return out
